# revision 33
# baseline (speedup 1.0000x reference)
"""Trainium2 Bass kernel for nn_DerivativeCalculator (Gauss-Newton gradient +
Hessian accumulation over N points per pose).

Contract: kernel(**inputs) takes the FULL inputs (as produced by
setup_inputs()) and returns the full output (gradient [B,6,1], hessian
[B,6,6]) as float32 numpy arrays, matching the reference.

Sharding: data-parallel over the pose dim B=64 across 8 NeuronCores
(8 poses per core). Per core, the 8 poses are laid out on the partition
axis (16 partitions per pose) with 512 points per partition on the free
axis. The per-point pipeline runs on VectorE/ScalarE/GpSimdE; the final
sum over the 8192 points per pose is done with fused multiply-reduce ops
(free axis) plus one TensorE matmul against a block selector (partition
axis), so no cross-core communication is needed.

Note: `distributions` ([B,N,200], ~420 MB) and `it` are unused by the
reference computation and are never transferred to the device.
"""

import numpy as np

B, N = 64, 8192
NCORES = 8
PB = B // NCORES          # poses per core = 8
PP = 16                   # partition groups per pose
F = N // PP               # free size = 512
NQ = 27                   # 6 gradient + 21 upper-triangular hessian sums

# pairs (i,j) for the hessian upper triangle, in emission order
HPAIRS = [(i, j) for i in range(6) for j in range(i, 6)]

# configuration: dtype 'f32'|'f16', reduce 'ttr'|'pe'
CFG = {"dtype": "f16", "reduce": "pe",
       # tuning knobs (TimelineSim-optimized)
       "n_prod_pool": 0,     # products computed on GpSimd instead of VE
       "n_cross_ve": 3,      # cross-product rows on VE instead of GpSimd
       "xy_eng": "ve",       # x/y row fused multiply-adds (one-wait rule)
       "dt_acts": True,      # d_dtrans rows via ScalarE scales + VE adds
       "c_pool": False}      # c1/c2 products on GpSimd

# set TRACE_KW to e.g. {"trace": True} before calling kernel() to profile;
# the raw BassKernelResults of the last run lands in LAST_RESULTS.
TRACE_KW = {}
LAST_RESULTS = None

_BUILT = {}


def _build(cfg):
    """Raw-bass (Block) program: this toolchain's walrus rejects TileContext's
    multi-wait drain, so synchronization is explicit — standalone wait_ge
    instructions plus .then_inc updates, each compute op carrying no inline
    waits."""
    import concourse.bass as bass
    import concourse.mybir as mybir
    from contextlib import ExitStack

    dt = mybir.dt
    Alu = mybir.AluOpType
    AF = mybir.ActivationFunctionType
    f32 = dt.float32
    assert cfg["dtype"] == "f16" and cfg["reduce"] == "pe"
    DT = dt.float16
    wscale = 2.0 ** -10

    nc = bass.Bass(trn_type="TRN2", detect_race_conditions=False)
    cstw, selw = 40, 32
    in1_d = nc.declare_dram_parameter("in1", [128, cstw + 3 * F], DT,
                                      isOutput=False)
    in2_d = nc.declare_dram_parameter("in2", [128, 7 * F + selw], DT,
                                      isOutput=False)
    out_d = nc.declare_dram_parameter("out", [128, 7], f32, isOutput=True)

    NPROD = 28
    with ExitStack() as ctx:
        E = ctx.enter_context

        def S16(name, w=F):
            return E(nc.sbuf_tensor(name, [128, w], DT))

        def S32(name, w=F):
            return E(nc.sbuf_tensor(name, [128, w], f32))

        blob1 = S16("blob1", cstw + 3 * F)
        blob2 = S16("blob2", 7 * F + selw)
        cst = blob1[:, 0:cstw].bitcast(f32)
        o = cstw
        px = blob1[:, o : o + F]
        py = blob1[:, o + F : o + 2 * F]
        pz = blob1[:, o + 2 * F : o + 3 * F]
        n0, n1 = blob2[:, 0:F], blob2[:, F : 2 * F]
        ciu, civ = blob2[:, 2 * F : 3 * F], blob2[:, 3 * F : 4 * F]
        mean_ = blob2[:, 4 * F : 5 * F]
        unc_ = blob2[:, 5 * F : 6 * F]
        msk = blob2[:, 6 * F : 7 * F]
        sel = blob2[:, 7 * F : 7 * F + selw]

        def c(i):
            return cst[:, i : i + 1]

        z = S16("z")
        r0 = S32("r0")
        zr = S32("zr")
        u2t = S32("u2t")
        zinv32 = S32("zinv32")
        zinv = S16("zinv")
        zinv2 = S16("zinv2")
        x = S16("x")
        y = S16("y")
        a = S16("a")
        b = S16("b")
        dxt = S16("dx")
        dyt = S16("dy")
        t1 = S16("t1")
        t2 = S16("t2")
        num = S16("num")
        dzp = S16("dzp")
        pd = S16("pd")
        eu = S16("eu")
        ev = S16("ev")
        c1t = S16("c1t")
        c2t = S16("c2t")
        md = S16("md")
        w0 = S16("w0")
        w = S16("w")
        gw = S16("gw")
        scr1 = S16("scr1")
        scr2 = S16("scr2")
        scr3 = S16("scr3")
        dts = [S16(f"dt{i}") for i in range(3)]
        crs = [S16(f"cr{i}") for i in range(3)]
        wth = [S16(f"wth{i}") for i in range(6)]
        prods = [S16(f"prod{k}") for k in range(NPROD)]
        ascr = [S16(f"ascr{v}") for v in range(7)]
        partials = S32("partials", 7)
        banks = [E(nc.psum_tensor(f"bank{v}", [128, F], f32))
                 for v in range(7)]

        sem_dma = E(nc.semaphore("sem_dma"))
        sem_ve = E(nc.semaphore("sem_ve"))      # VE milestones: z=1, prods 2..
        sem_act = E(nc.semaphore("sem_act"))    # r0=1, reduces 2..8
        sem_pe = E(nc.semaphore("sem_pe"))      # one per matmul

        theta = [crs[0], crs[1], crs[2], dts[0], dts[1], dts[2]]
        pr_pairs = [(gw, theta[i]) for i in range(6)] + [
            (wth[i], theta[j]) for (i, j) in HPAIRS
        ]
        pr_pairs.append(pr_pairs[0])  # pad to 28 so every bank fills

        blk = E(nc.Block())

        @blk.sync
        def _(sync):
            sync.dma_start(blob1[:], in1_d[:]).then_inc(sem_dma, 16)
            sync.dma_start(blob2[:], in2_d[:]).then_inc(sem_dma, 16)
            sync.wait_ge(sem_act, 8)
            sync.dma_start(out_d[:], partials[:]).then_inc(sem_dma, 16)

        @blk.vector
        def _(vector):
            V = nc.vector

            def lin3(out, i0, i1, i2, k0, k1, k2, kt):
                # out = i0*c(k0) + i1*c(k1) + i2*c(k2) + c(kt)
                V.tensor_scalar(scr1[:], i0, c(k0), None, Alu.mult)
                V.tensor_scalar(scr2[:], i1, c(k1), None, Alu.mult)
                V.tensor_tensor(scr3[:], scr1[:], scr2[:], Alu.add)
                V.tensor_scalar(scr1[:], i2, c(k2), c(kt), Alu.mult, Alu.add)
                V.tensor_tensor(out, scr3[:], scr1[:], Alu.add)

            vector.wait_ge(sem_dma, 16)
            # z row first: feeds the ScalarE reciprocal
            lin3(z[:], px, py, pz, 6, 7, 8, 14)
            nc.vector.engine_nop().then_inc(sem_ve, 1)
            # x/y rows exclude +t (folded into t1/t2 below)
            V.tensor_scalar(scr1[:], px, c(0), None, Alu.mult)
            V.tensor_scalar(scr2[:], py, c(1), None, Alu.mult)
            V.tensor_tensor(scr3[:], scr1[:], scr2[:], Alu.add)
            V.tensor_scalar(scr1[:], pz, c(2), None, Alu.mult)
            V.tensor_tensor(x[:], scr3[:], scr1[:], Alu.add)
            V.tensor_scalar(scr1[:], px, c(3), None, Alu.mult)
            V.tensor_scalar(scr2[:], py, c(4), None, Alu.mult)
            V.tensor_tensor(scr3[:], scr1[:], scr2[:], Alu.add)
            V.tensor_scalar(scr1[:], pz, c(5), None, Alu.mult)
            V.tensor_tensor(y[:], scr3[:], scr1[:], Alu.add)

            vector.wait_ge(sem_dma, 32)
            V.tensor_scalar(a[:], n0, c(15), None, Alu.mult)
            V.tensor_scalar(b[:], n1, c(16), None, Alu.mult)
            V.tensor_scalar(eu[:], ciu, -1.0, c(17), Alu.mult, Alu.add)
            V.tensor_scalar(ev[:], civ, -1.0, c(18), Alu.mult, Alu.add)
            V.tensor_tensor(c1t[:], n0, eu[:], Alu.mult)
            V.tensor_tensor(c2t[:], n1, ev[:], Alu.mult)
            V.tensor_scalar(w0[:], unc_, float(wscale), None, Alu.mult)
            V.tensor_tensor(w[:], w0[:], msk, Alu.mult)

            # Newton polish of the ScalarE spline reciprocal
            vector.wait_ge(sem_act, 1)
            V.tensor_tensor(zr[:], z[:], r0[:], Alu.mult)
            V.tensor_scalar(u2t[:], zr[:], -1.0, 2.0, Alu.mult, Alu.add)
            V.tensor_tensor(zinv32[:], r0[:], u2t[:], Alu.mult)
            V.tensor_copy(zinv[:], zinv32[:])
            V.tensor_tensor(zinv2[:], zinv[:], zinv[:], Alu.mult)

            V.tensor_tensor(dxt[:], a[:], zinv[:], Alu.mult)
            V.tensor_tensor(dyt[:], b[:], zinv[:], Alu.mult)
            V.tensor_tensor(t1[:], a[:], x[:], Alu.mult)
            V.tensor_scalar(scr1[:], a[:], c(12), None, Alu.mult)
            V.tensor_tensor(t1[:], t1[:], scr1[:], Alu.add)
            V.tensor_tensor(t2[:], b[:], y[:], Alu.mult)
            V.tensor_scalar(scr2[:], b[:], c(13), None, Alu.mult)
            V.tensor_tensor(t2[:], t2[:], scr2[:], Alu.add)
            V.tensor_tensor(num[:], t1[:], t2[:], Alu.add)
            V.tensor_tensor(dzp[:], num[:], zinv2[:], Alu.mult)
            V.tensor_tensor(pd[:], num[:], zinv[:], Alu.mult)
            V.tensor_tensor(md[:], mean_, pd[:], Alu.subtract)
            V.tensor_tensor(md[:], md[:], c1t[:], Alu.subtract)
            V.tensor_tensor(md[:], md[:], c2t[:], Alu.subtract)
            V.tensor_tensor(gw[:], md[:], w[:], Alu.mult)

            for i in range(3):
                V.tensor_scalar(scr1[:], dxt[:], c(i), None, Alu.mult)
                V.tensor_scalar(scr2[:], dyt[:], c(3 + i), None, Alu.mult)
                V.tensor_tensor(scr3[:], scr1[:], scr2[:], Alu.add)
                V.tensor_scalar(scr1[:], dzp[:], c(9 + i), None, Alu.mult)
                V.tensor_tensor(dts[i][:], scr3[:], scr1[:], Alu.add)

            for (ri, ia, ib, pa, pb_) in (
                (0, 2, 1, py, pz),
                (1, 0, 2, pz, px),
                (2, 1, 0, px, py),
            ):
                V.tensor_tensor(scr1[:], pa, dts[ia][:], Alu.mult)
                V.tensor_tensor(scr2[:], pb_, dts[ib][:], Alu.mult)
                V.tensor_tensor(crs[ri][:], scr1[:], scr2[:], Alu.subtract)

            for i in range(6):
                V.tensor_tensor(wth[i][:], w[:], theta[i][:], Alu.mult)

            for k, (ta, tb) in enumerate(pr_pairs):
                V.tensor_tensor(
                    prods[k][:], ta[:], tb[:], Alu.mult
                ).then_inc(sem_ve, 1)

        @blk.scalar
        def _(scalar):
            S = nc.scalar
            scalar.wait_ge(sem_ve, 1)
            S.add_instruction(
                mybir.InstActivation(
                    name=nc.get_next_instruction_name(),
                    func=AF.Reciprocal,
                    ins=[
                        S.lower_ap(z[:]),
                        mybir.ImmediateValue(dtype=f32, value=0.0),
                        mybir.ImmediateValue(dtype=f32, value=1.0),
                        mybir.ImmediateValue(dtype=f32, value=0.0),
                    ],
                    outs=[S.lower_ap(r0[:])],
                )
            ).then_inc(sem_act, 1)
            for v in range(7):
                scalar.wait_ge(sem_pe, 4 * v + 4)
                S.activation(
                    ascr[v][:], banks[v][:], AF.Copy,
                    accum_out=partials[:, v : v + 1],
                ).then_inc(sem_act, 1)

        @blk.tensor
        def _(tensor):
            tensor.wait_ge(sem_dma, 32)
            for k in range(NPROD):
                tensor.wait_ge(sem_ve, 2 + k)
                wv, j = k // 4, k % 4
                nc.tensor.matmul(
                    banks[wv][32 * j : 32 * j + 32, :],
                    sel,
                    prods[k][:],
                    start=True,
                    stop=True,
                    tile_position=(0, 32 * j),
                ).then_inc(sem_pe, 1)

    return nc


def _get_nc(cfg_key):
    if cfg_key not in _BUILT:
        _BUILT[cfg_key] = _build(CFG)
    return _BUILT[cfg_key]


def _pack_core_inputs(c, normals_in_image, centers_in_image, centers_in_body,
                      deformed_body2view_pose_data, camera_data,
                      valid_data_line, distribution_mean,
                      distribution_uncertainties, np_dt):
    sl = slice(PB * c, PB * (c + 1))

    def pack(arr2d):  # [PB, N] -> [128, F]
        return np.ascontiguousarray(
            arr2d.reshape(PB * PP, F), dtype=np_dt
        )

    p3 = centers_in_body[sl]
    nrm = normals_in_image[sl]
    cim = centers_in_image[sl]
    p3_a = np.concatenate([pack(p3[..., i]) for i in range(3)], axis=1)
    nrm_a = np.concatenate([pack(nrm[..., i]) for i in range(2)], axis=1)
    cim_a = np.concatenate([pack(cim[..., i]) for i in range(2)], axis=1)
    stat_a = np.concatenate(
        [
            pack(distribution_mean[sl]),
            pack(distribution_uncertainties[sl]),
            pack(valid_data_line[sl].astype(np.float32)),
        ],
        axis=1,
    )

    pose = deformed_body2view_pose_data[sl].astype(np.float32)  # [PB,12]
    cam = camera_data[sl].astype(np.float32)  # [PB,6]
    R = pose[:, :9]
    cstv = np.concatenate(
        [
            R,                               # 0..8
            -R[:, 6:9],                      # 9..11  (-R[2,:])
            pose[:, 9:12],                   # 12..14 (t)
            cam[:, 2:3], cam[:, 3:4],        # 15, 16 (fu, fv)
            cam[:, 4:5], cam[:, 5:6],        # 17, 18 (cu, cv)
            np.zeros((PB, 1), np.float32),   # 19 (zero bias)
        ],
        axis=1,
    )  # [PB, 20]
    cst_a = np.ascontiguousarray(np.repeat(cstv, PP, axis=0), dtype=np.float32)

    selw = PB if CFG["reduce"] != "pe" else 32
    sel_np = np_dt if CFG["reduce"] == "pe" else np.float32
    sel_a = np.zeros((128, selw), sel_np)
    for j in range(selw):
        sel_a[PP * (j % PB) : PP * (j % PB + 1), j] = 1.0

    if np_dt == np.float16:
        cst_cols = np.ascontiguousarray(cst_a).view(np.float16)
    else:
        cst_cols = cst_a
    in1 = np.concatenate([cst_cols.astype(cst_cols.dtype), p3_a], axis=1)
    sel_cols = sel_a
    if np_dt == np.float16 and sel_a.dtype == np.float32:
        sel_cols = np.ascontiguousarray(sel_a).view(np.float16)
    in2 = np.concatenate([nrm_a, cim_a, stat_a, sel_cols], axis=1)
    return {"in1": np.ascontiguousarray(in1),
            "in2": np.ascontiguousarray(in2)}


def _decode(results, cfg):
    wscale = 2.0 ** -10 if cfg["dtype"] == "f16" else 1.0
    parts = np.zeros((B, NQ), np.float32)
    for cidx in range(NCORES):
        out = np.asarray(results[cidx]["out"], np.float32)
        if cfg["reduce"] == "pe":
            for q in range(NQ):
                wv, j = q // 4, q % 4
                parts[PB * cidx : PB * (cidx + 1), q] = out[
                    32 * j : 32 * j + PB, wv
                ]
        else:
            parts[PB * cidx : PB * (cidx + 1), :] = out
    parts /= wscale
    grad = np.ascontiguousarray(parts[:, :6, None], np.float32)
    hess = np.zeros((B, 6, 6), np.float32)
    for idx, (i, j) in enumerate(HPAIRS):
        hess[:, i, j] = parts[:, 6 + idx]
        hess[:, j, i] = parts[:, 6 + idx]
    return grad, hess


def kernel(normals_in_image, centers_in_image, centers_in_body,
           deformed_body2view_pose_data, camera_data, valid_data_line,
           distributions=None, distribution_mean=None,
           distribution_uncertainties=None, it=None, **_unused):
    from concourse.bass_utils import run_bass_kernel_spmd

    cfg = CFG
    np_dt = np.float32 if cfg["dtype"] == "f32" else np.float16
    nc = _get_nc((cfg["dtype"], cfg["reduce"]))

    args = (
        np.asarray(normals_in_image),
        np.asarray(centers_in_image),
        np.asarray(centers_in_body),
        np.asarray(deformed_body2view_pose_data),
        np.asarray(camera_data),
        np.asarray(valid_data_line),
        np.asarray(distribution_mean),
        np.asarray(distribution_uncertainties),
    )
    in_maps = [_pack_core_inputs(c, *args, np_dt) for c in range(NCORES)]
    res = run_bass_kernel_spmd(
        nc, in_maps, core_ids=list(range(NCORES)), **TRACE_KW
    )
    global LAST_RESULTS
    LAST_RESULTS = res
    return _decode(res.results, cfg)


# revision 39
# speedup vs baseline: 1.0332x; 1.0332x over previous
"""Trainium2 Bass kernel for nn_DerivativeCalculator (Gauss-Newton gradient +
Hessian accumulation over N points per pose).

Contract: kernel(**inputs) takes the FULL inputs (as produced by
setup_inputs()) and returns the full output (gradient [B,6,1], hessian
[B,6,6]) as float32 numpy arrays, matching the reference.

Sharding: data-parallel over the pose dim B=64 across 8 NeuronCores
(8 poses per core). Per core, the 8 poses are laid out on the partition
axis (16 partitions per pose) with 512 points per partition on the free
axis. The per-point pipeline runs on VectorE/ScalarE/GpSimdE; the final
sum over the 8192 points per pose is done with fused multiply-reduce ops
(free axis) plus one TensorE matmul against a block selector (partition
axis), so no cross-core communication is needed.

Note: `distributions` ([B,N,200], ~420 MB) and `it` are unused by the
reference computation and are never transferred to the device.
"""

import numpy as np

B, N = 64, 8192
NCORES = 8
PB = B // NCORES          # poses per core = 8
PP = 16                   # partition groups per pose
F = N // PP               # free size = 512
NQ = 27                   # 6 gradient + 21 upper-triangular hessian sums

# pairs (i,j) for the hessian upper triangle, in emission order
HPAIRS = [(i, j) for i in range(6) for j in range(i, 6)]

# configuration: dtype 'f32'|'f16', reduce 'ttr'|'pe'
CFG = {"dtype": "f16", "reduce": "pe",
       # tuning knobs (TimelineSim-optimized)
       "n_prod_pool": 0,     # products computed on GpSimd instead of VE
       "n_cross_ve": 3,      # cross-product rows on VE instead of GpSimd
       "xy_eng": "ve",       # x/y row fused multiply-adds (one-wait rule)
       "dt_acts": True,      # d_dtrans rows via ScalarE scales + VE adds
       "c_pool": False}      # c1/c2 products on GpSimd

# set TRACE_KW to e.g. {"trace": True} before calling kernel() to profile;
# the raw BassKernelResults of the last run lands in LAST_RESULTS.
TRACE_KW = {}
LAST_RESULTS = None

_BUILT = {}


def _build(cfg):
    """Raw-bass (Block) program: this toolchain's walrus rejects TileContext's
    multi-wait drain, so synchronization is explicit — standalone wait_ge
    instructions plus .then_inc updates, each compute op carrying no inline
    waits."""
    import concourse.bass as bass
    import concourse.mybir as mybir
    from contextlib import ExitStack

    dt = mybir.dt
    Alu = mybir.AluOpType
    AF = mybir.ActivationFunctionType
    f32 = dt.float32
    assert cfg["dtype"] == "f16" and cfg["reduce"] == "pe"
    DT = dt.float16
    wscale = 2.0 ** -10

    nc = bass.Bass(trn_type="TRN2", detect_race_conditions=False)
    cstw, selw = 40, 32
    in1_d = nc.declare_dram_parameter("in1", [128, cstw + 3 * F], DT,
                                      isOutput=False)
    in2_d = nc.declare_dram_parameter("in2", [128, 7 * F + selw], DT,
                                      isOutput=False)
    out_d = nc.declare_dram_parameter("out", [128, 7], f32, isOutput=True)

    NPROD = 28
    with ExitStack() as ctx:
        E = ctx.enter_context

        def S16(name, w=F):
            return E(nc.sbuf_tensor(name, [128, w], DT))

        def S32(name, w=F):
            return E(nc.sbuf_tensor(name, [128, w], f32))

        blob1 = S16("blob1", cstw + 3 * F)
        blob2 = S16("blob2", 7 * F + selw)
        cst = blob1[:, 0:cstw].bitcast(f32)
        o = cstw
        px = blob1[:, o : o + F]
        py = blob1[:, o + F : o + 2 * F]
        pz = blob1[:, o + 2 * F : o + 3 * F]
        n0, n1 = blob2[:, 0:F], blob2[:, F : 2 * F]
        ciu, civ = blob2[:, 2 * F : 3 * F], blob2[:, 3 * F : 4 * F]
        mean_ = blob2[:, 4 * F : 5 * F]
        unc_ = blob2[:, 5 * F : 6 * F]
        msk = blob2[:, 6 * F : 7 * F]
        sel = blob2[:, 7 * F : 7 * F + selw]

        def c(i):
            return cst[:, i : i + 1]

        z = S16("z")
        r0 = S32("r0")
        zr = S32("zr")
        u2t = S32("u2t")
        zinv32 = S32("zinv32")
        zinv = S16("zinv")
        zinv2 = S16("zinv2")
        x = S16("x")
        y = S16("y")
        a = S16("a")
        b = S16("b")
        dxt = S16("dx")
        dyt = S16("dy")
        t1 = S16("t1")
        t2 = S16("t2")
        num = S16("num")
        dzp = S16("dzp")
        pd = S16("pd")
        eu = S16("eu")
        ev = S16("ev")
        c1t = S16("c1t")
        c2t = S16("c2t")
        md = S16("md")
        w0 = S16("w0")
        w = S16("w")
        gw = S16("gw")
        scr1 = S16("scr1")
        scr2 = S16("scr2")
        scr3 = S16("scr3")
        dts = [S16(f"dt{i}") for i in range(3)]
        crs = [S16(f"cr{i}") for i in range(3)]
        wth = [S16(f"wth{i}") for i in range(6)]
        prods = [S16(f"prod{k}") for k in range(NPROD)]
        ascr = [S16(f"ascr{v}") for v in range(7)]
        partials = S32("partials", 7)
        banks = [E(nc.psum_tensor(f"bank{v}", [128, F], f32))
                 for v in range(7)]

        sem_dma = E(nc.semaphore("sem_dma"))
        sem_ve = E(nc.semaphore("sem_ve"))      # milestones: z=1 eu/ev=2 wth=3
        sem_vp = E(nc.semaphore("sem_vp"))      # VE product stream
        sem_gp = E(nc.semaphore("sem_gp"))      # GP: c1=1 c2=2 products 3..8
        sem_act = E(nc.semaphore("sem_act"))    # r0=1, reduces 2..8
        sem_pe = E(nc.semaphore("sem_pe"))      # one per matmul

        theta = [crs[0], crs[1], crs[2], dts[0], dts[1], dts[2]]
        pr_pairs = [(gw, theta[i]) for i in range(6)] + [
            (wth[i], theta[j]) for (i, j) in HPAIRS
        ]
        pr_pairs.append(pr_pairs[0])  # pad to 28 so every bank fills
        GPK = [21, 22, 23, 24, 25, 26]   # dt-only hessian pairs -> GpSimd
        VEK = [k for k in range(NPROD) if k not in GPK]

        blk = E(nc.Block())

        @blk.sync
        def _(sync):
            sync.dma_start(blob1[:], in1_d[:]).then_inc(sem_dma, 16)
            sync.dma_start(blob2[:], in2_d[:]).then_inc(sem_dma, 16)
            sync.wait_ge(sem_act, 8)
            sync.dma_start(out_d[:], partials[:]).then_inc(sem_dma, 16)

        @blk.vector
        def _(vector):
            V = nc.vector

            def lin3(out, i0, i1, i2, k0, k1, k2, kt):
                # out = i0*c(k0) + i1*c(k1) + i2*c(k2) + c(kt)
                V.tensor_scalar(scr1[:], i0, c(k0), None, Alu.mult)
                V.tensor_scalar(scr2[:], i1, c(k1), None, Alu.mult)
                V.tensor_tensor(scr3[:], scr1[:], scr2[:], Alu.add)
                V.tensor_scalar(scr1[:], i2, c(k2), c(kt), Alu.mult, Alu.add)
                V.tensor_tensor(out, scr3[:], scr1[:], Alu.add)

            vector.wait_ge(sem_dma, 16)
            # z row first: feeds the ScalarE reciprocal
            lin3(z[:], px, py, pz, 6, 7, 8, 14)
            nc.vector.engine_nop().then_inc(sem_ve, 1)
            # x/y rows exclude +t (folded into t1/t2 below)
            V.tensor_scalar(scr1[:], px, c(0), None, Alu.mult)
            V.tensor_scalar(scr2[:], py, c(1), None, Alu.mult)
            V.tensor_tensor(scr3[:], scr1[:], scr2[:], Alu.add)
            V.tensor_scalar(scr1[:], pz, c(2), None, Alu.mult)
            V.tensor_tensor(x[:], scr3[:], scr1[:], Alu.add)
            V.tensor_scalar(scr1[:], px, c(3), None, Alu.mult)
            V.tensor_scalar(scr2[:], py, c(4), None, Alu.mult)
            V.tensor_tensor(scr3[:], scr1[:], scr2[:], Alu.add)
            V.tensor_scalar(scr1[:], pz, c(5), None, Alu.mult)
            V.tensor_tensor(y[:], scr3[:], scr1[:], Alu.add)

            vector.wait_ge(sem_dma, 32)
            V.tensor_scalar(a[:], n0, c(15), None, Alu.mult)
            V.tensor_scalar(b[:], n1, c(16), None, Alu.mult)
            V.tensor_scalar(eu[:], ciu, -1.0, c(17), Alu.mult, Alu.add)
            V.tensor_scalar(
                ev[:], civ, -1.0, c(18), Alu.mult, Alu.add
            ).then_inc(sem_ve, 1)  # =2: eu/ev ready for GpSimd c1/c2
            V.tensor_scalar(w0[:], unc_, float(wscale), None, Alu.mult)
            V.tensor_tensor(w[:], w0[:], msk, Alu.mult)

            # Newton polish of the ScalarE spline reciprocal
            vector.wait_ge(sem_act, 1)
            V.tensor_tensor(zr[:], z[:], r0[:], Alu.mult)
            V.tensor_scalar(u2t[:], zr[:], -1.0, 2.0, Alu.mult, Alu.add)
            V.tensor_tensor(zinv32[:], r0[:], u2t[:], Alu.mult)
            V.tensor_copy(zinv[:], zinv32[:])
            V.tensor_tensor(zinv2[:], zinv[:], zinv[:], Alu.mult)

            V.tensor_tensor(dxt[:], a[:], zinv[:], Alu.mult)
            V.tensor_tensor(dyt[:], b[:], zinv[:], Alu.mult)
            V.tensor_tensor(t1[:], a[:], x[:], Alu.mult)
            V.tensor_scalar(scr1[:], a[:], c(12), None, Alu.mult)
            V.tensor_tensor(t1[:], t1[:], scr1[:], Alu.add)
            V.tensor_tensor(t2[:], b[:], y[:], Alu.mult)
            V.tensor_scalar(scr2[:], b[:], c(13), None, Alu.mult)
            V.tensor_tensor(t2[:], t2[:], scr2[:], Alu.add)
            V.tensor_tensor(num[:], t1[:], t2[:], Alu.add)
            V.tensor_tensor(dzp[:], num[:], zinv2[:], Alu.mult)
            V.tensor_tensor(pd[:], num[:], zinv[:], Alu.mult)
            V.tensor_tensor(md[:], mean_, pd[:], Alu.subtract)
            vector.wait_ge(sem_gp, 2)   # c1t/c2t from GpSimd
            V.tensor_tensor(md[:], md[:], c1t[:], Alu.subtract)
            V.tensor_tensor(md[:], md[:], c2t[:], Alu.subtract)
            V.tensor_tensor(gw[:], md[:], w[:], Alu.mult)

            for i in range(3):
                V.tensor_scalar(scr1[:], dxt[:], c(i), None, Alu.mult)
                V.tensor_scalar(scr2[:], dyt[:], c(3 + i), None, Alu.mult)
                V.tensor_tensor(scr3[:], scr1[:], scr2[:], Alu.add)
                V.tensor_scalar(scr1[:], dzp[:], c(9 + i), None, Alu.mult)
                V.tensor_tensor(dts[i][:], scr3[:], scr1[:], Alu.add)

            for i in range(3, 6):
                V.tensor_tensor(wth[i][:], w[:], theta[i][:], Alu.mult)
            nc.vector.engine_nop().then_inc(sem_ve, 1)  # =3: dt wth ready

            for (ri, ia, ib, pa, pb_) in (
                (0, 2, 1, py, pz),
                (1, 0, 2, pz, px),
                (2, 1, 0, px, py),
            ):
                V.tensor_tensor(scr1[:], pa, dts[ia][:], Alu.mult)
                V.tensor_tensor(scr2[:], pb_, dts[ib][:], Alu.mult)
                V.tensor_tensor(crs[ri][:], scr1[:], scr2[:], Alu.subtract)

            for i in range(3):
                V.tensor_tensor(wth[i][:], w[:], theta[i][:], Alu.mult)

            for k in VEK:
                ta, tb = pr_pairs[k]
                V.tensor_tensor(
                    prods[k][:], ta[:], tb[:], Alu.mult
                ).then_inc(sem_vp, 1)

        @blk.scalar
        def _(scalar):
            S = nc.scalar
            scalar.wait_ge(sem_ve, 1)
            S.add_instruction(
                mybir.InstActivation(
                    name=nc.get_next_instruction_name(),
                    func=AF.Reciprocal,
                    ins=[
                        S.lower_ap(z[:]),
                        mybir.ImmediateValue(dtype=f32, value=0.0),
                        mybir.ImmediateValue(dtype=f32, value=1.0),
                        mybir.ImmediateValue(dtype=f32, value=0.0),
                    ],
                    outs=[S.lower_ap(r0[:])],
                )
            ).then_inc(sem_act, 1)
            for v in range(7):
                scalar.wait_ge(sem_pe, 4 * v + 4)
                S.activation(
                    ascr[v][:], banks[v][:], AF.Copy,
                    accum_out=partials[:, v : v + 1],
                ).then_inc(sem_act, 1)

        @blk.gpsimd
        def _(gpsimd):
            G = nc.gpsimd
            gpsimd.wait_ge(sem_dma, 32)
            gpsimd.wait_ge(sem_ve, 2)
            G.tensor_tensor(c1t[:], n0, eu[:], Alu.mult).then_inc(sem_gp, 1)
            G.tensor_tensor(c2t[:], n1, ev[:], Alu.mult).then_inc(sem_gp, 1)
            gpsimd.wait_ge(sem_ve, 3)
            for k in GPK:
                ta, tb = pr_pairs[k]
                G.tensor_tensor(
                    prods[k][:], ta[:], tb[:], Alu.mult
                ).then_inc(sem_gp, 1)

        @blk.tensor
        def _(tensor):
            tensor.wait_ge(sem_dma, 32)
            ve_pos = {k: i + 1 for i, k in enumerate(VEK)}
            gp_pos = {k: i + 3 for i, k in enumerate(GPK)}
            for k in range(NPROD):
                if k in gp_pos:
                    tensor.wait_ge(sem_gp, gp_pos[k])
                else:
                    tensor.wait_ge(sem_vp, ve_pos[k])
                wv, j = k // 4, k % 4
                nc.tensor.matmul(
                    banks[wv][32 * j : 32 * j + 32, :],
                    sel,
                    prods[k][:],
                    start=True,
                    stop=True,
                    tile_position=(0, 32 * j),
                ).then_inc(sem_pe, 1)

    return nc


def _get_nc(cfg_key):
    if cfg_key not in _BUILT:
        _BUILT[cfg_key] = _build(CFG)
    return _BUILT[cfg_key]


def _pack_core_inputs(c, normals_in_image, centers_in_image, centers_in_body,
                      deformed_body2view_pose_data, camera_data,
                      valid_data_line, distribution_mean,
                      distribution_uncertainties, np_dt):
    sl = slice(PB * c, PB * (c + 1))

    def pack(arr2d):  # [PB, N] -> [128, F]
        return np.ascontiguousarray(
            arr2d.reshape(PB * PP, F), dtype=np_dt
        )

    p3 = centers_in_body[sl]
    nrm = normals_in_image[sl]
    cim = centers_in_image[sl]
    p3_a = np.concatenate([pack(p3[..., i]) for i in range(3)], axis=1)
    nrm_a = np.concatenate([pack(nrm[..., i]) for i in range(2)], axis=1)
    cim_a = np.concatenate([pack(cim[..., i]) for i in range(2)], axis=1)
    stat_a = np.concatenate(
        [
            pack(distribution_mean[sl]),
            pack(distribution_uncertainties[sl]),
            pack(valid_data_line[sl].astype(np.float32)),
        ],
        axis=1,
    )

    pose = deformed_body2view_pose_data[sl].astype(np.float32)  # [PB,12]
    cam = camera_data[sl].astype(np.float32)  # [PB,6]
    R = pose[:, :9]
    cstv = np.concatenate(
        [
            R,                               # 0..8
            -R[:, 6:9],                      # 9..11  (-R[2,:])
            pose[:, 9:12],                   # 12..14 (t)
            cam[:, 2:3], cam[:, 3:4],        # 15, 16 (fu, fv)
            cam[:, 4:5], cam[:, 5:6],        # 17, 18 (cu, cv)
            np.zeros((PB, 1), np.float32),   # 19 (zero bias)
        ],
        axis=1,
    )  # [PB, 20]
    cst_a = np.ascontiguousarray(np.repeat(cstv, PP, axis=0), dtype=np.float32)

    selw = PB if CFG["reduce"] != "pe" else 32
    sel_np = np_dt if CFG["reduce"] == "pe" else np.float32
    sel_a = np.zeros((128, selw), sel_np)
    for j in range(selw):
        sel_a[PP * (j % PB) : PP * (j % PB + 1), j] = 1.0

    if np_dt == np.float16:
        cst_cols = np.ascontiguousarray(cst_a).view(np.float16)
    else:
        cst_cols = cst_a
    in1 = np.concatenate([cst_cols.astype(cst_cols.dtype), p3_a], axis=1)
    sel_cols = sel_a
    if np_dt == np.float16 and sel_a.dtype == np.float32:
        sel_cols = np.ascontiguousarray(sel_a).view(np.float16)
    in2 = np.concatenate([nrm_a, cim_a, stat_a, sel_cols], axis=1)
    return {"in1": np.ascontiguousarray(in1),
            "in2": np.ascontiguousarray(in2)}


def _decode(results, cfg):
    wscale = 2.0 ** -10 if cfg["dtype"] == "f16" else 1.0
    parts = np.zeros((B, NQ), np.float32)
    for cidx in range(NCORES):
        out = np.asarray(results[cidx]["out"], np.float32)
        if cfg["reduce"] == "pe":
            for q in range(NQ):
                wv, j = q // 4, q % 4
                parts[PB * cidx : PB * (cidx + 1), q] = out[
                    32 * j : 32 * j + PB, wv
                ]
        else:
            parts[PB * cidx : PB * (cidx + 1), :] = out
    parts /= wscale
    grad = np.ascontiguousarray(parts[:, :6, None], np.float32)
    hess = np.zeros((B, 6, 6), np.float32)
    for idx, (i, j) in enumerate(HPAIRS):
        hess[:, i, j] = parts[:, 6 + idx]
        hess[:, j, i] = parts[:, 6 + idx]
    return grad, hess


def kernel(normals_in_image, centers_in_image, centers_in_body,
           deformed_body2view_pose_data, camera_data, valid_data_line,
           distributions=None, distribution_mean=None,
           distribution_uncertainties=None, it=None, **_unused):
    from concourse.bass_utils import run_bass_kernel_spmd

    cfg = CFG
    np_dt = np.float32 if cfg["dtype"] == "f32" else np.float16
    nc = _get_nc((cfg["dtype"], cfg["reduce"]))

    args = (
        np.asarray(normals_in_image),
        np.asarray(centers_in_image),
        np.asarray(centers_in_body),
        np.asarray(deformed_body2view_pose_data),
        np.asarray(camera_data),
        np.asarray(valid_data_line),
        np.asarray(distribution_mean),
        np.asarray(distribution_uncertainties),
    )
    in_maps = [_pack_core_inputs(c, *args, np_dt) for c in range(NCORES)]
    res = run_bass_kernel_spmd(
        nc, in_maps, core_ids=list(range(NCORES)), **TRACE_KW
    )
    global LAST_RESULTS
    LAST_RESULTS = res
    return _decode(res.results, cfg)


# revision 40
# speedup vs baseline: 1.0729x; 1.0384x over previous
"""Trainium2 Bass kernel for nn_DerivativeCalculator (Gauss-Newton gradient +
Hessian accumulation over N points per pose).

Contract: kernel(**inputs) takes the FULL inputs (as produced by
setup_inputs()) and returns the full output (gradient [B,6,1], hessian
[B,6,6]) as float32 numpy arrays, matching the reference.

Sharding: data-parallel over the pose dim B=64 across 8 NeuronCores
(8 poses per core). Per core, the 8 poses are laid out on the partition
axis (16 partitions per pose) with 512 points per partition on the free
axis. The per-point pipeline runs on VectorE/ScalarE/GpSimdE; the final
sum over the 8192 points per pose is done with fused multiply-reduce ops
(free axis) plus one TensorE matmul against a block selector (partition
axis), so no cross-core communication is needed.

Note: `distributions` ([B,N,200], ~420 MB) and `it` are unused by the
reference computation and are never transferred to the device.
"""

import numpy as np

B, N = 64, 8192
NCORES = 8
PB = B // NCORES          # poses per core = 8
PP = 16                   # partition groups per pose
F = N // PP               # free size = 512
NQ = 27                   # 6 gradient + 21 upper-triangular hessian sums

# pairs (i,j) for the hessian upper triangle, in emission order
HPAIRS = [(i, j) for i in range(6) for j in range(i, 6)]

# configuration: dtype 'f32'|'f16', reduce 'ttr'|'pe'
CFG = {"dtype": "f16", "reduce": "pe",
       # tuning knobs (TimelineSim-optimized)
       "n_prod_pool": 0,     # products computed on GpSimd instead of VE
       "n_cross_ve": 3,      # cross-product rows on VE instead of GpSimd
       "xy_eng": "ve",       # x/y row fused multiply-adds (one-wait rule)
       "dt_acts": True,      # d_dtrans rows via ScalarE scales + VE adds
       "c_pool": False}      # c1/c2 products on GpSimd

# set TRACE_KW to e.g. {"trace": True} before calling kernel() to profile;
# the raw BassKernelResults of the last run lands in LAST_RESULTS.
TRACE_KW = {}
LAST_RESULTS = None

_BUILT = {}


def _build(cfg):
    """Raw-bass (Block) program: this toolchain's walrus rejects TileContext's
    multi-wait drain, so synchronization is explicit — standalone wait_ge
    instructions plus .then_inc updates, each compute op carrying no inline
    waits."""
    import concourse.bass as bass
    import concourse.mybir as mybir
    from contextlib import ExitStack

    dt = mybir.dt
    Alu = mybir.AluOpType
    AF = mybir.ActivationFunctionType
    f32 = dt.float32
    assert cfg["dtype"] == "f16" and cfg["reduce"] == "pe"
    DT = dt.float16
    wscale = 2.0 ** -10

    nc = bass.Bass(trn_type="TRN2", detect_race_conditions=False)
    cstw, selw = 40, 32
    in1_d = nc.declare_dram_parameter("in1", [128, cstw + 3 * F], DT,
                                      isOutput=False)
    in2_d = nc.declare_dram_parameter("in2", [128, 7 * F + selw], DT,
                                      isOutput=False)
    out_d = nc.declare_dram_parameter("out", [128, 7], f32, isOutput=True)

    NPROD = 28
    with ExitStack() as ctx:
        E = ctx.enter_context

        def S16(name, w=F):
            return E(nc.sbuf_tensor(name, [128, w], DT))

        def S32(name, w=F):
            return E(nc.sbuf_tensor(name, [128, w], f32))

        blob1 = S16("blob1", cstw + 3 * F)
        blob2 = S16("blob2", 7 * F + selw)
        cst = blob1[:, 0:cstw].bitcast(f32)
        o = cstw
        px = blob1[:, o : o + F]
        py = blob1[:, o + F : o + 2 * F]
        pz = blob1[:, o + 2 * F : o + 3 * F]
        n0, n1 = blob2[:, 0:F], blob2[:, F : 2 * F]
        ciu, civ = blob2[:, 2 * F : 3 * F], blob2[:, 3 * F : 4 * F]
        mean_ = blob2[:, 4 * F : 5 * F]
        unc_ = blob2[:, 5 * F : 6 * F]
        msk = blob2[:, 6 * F : 7 * F]
        sel = blob2[:, 7 * F : 7 * F + selw]

        def c(i):
            return cst[:, i : i + 1]

        z = S16("z")
        r0 = S32("r0")
        zr = S32("zr")
        u2t = S32("u2t")
        zinv32 = S32("zinv32")
        zinv = S16("zinv")
        zinv2 = S16("zinv2")
        x = S16("x")
        y = S16("y")
        a = S16("a")
        b = S16("b")
        dxt = S16("dx")
        dyt = S16("dy")
        t1 = S16("t1")
        t2 = S16("t2")
        num = S16("num")
        dzp = S16("dzp")
        pd = S16("pd")
        eu = S16("eu")
        ev = S16("ev")
        c1t = S16("c1t")
        c2t = S16("c2t")
        md = S16("md")
        w0 = S16("w0")
        w = S16("w")
        gw = S16("gw")
        scr1 = S16("scr1")
        scr2 = S16("scr2")
        scr3 = S16("scr3")
        gscr = S16("gscr")
        dts = [S16(f"dt{i}") for i in range(3)]
        crs = [S16(f"cr{i}") for i in range(3)]
        wth = [S16(f"wth{i}") for i in range(6)]
        prods = [S16(f"prod{k}") for k in range(NPROD)]
        ascr = [S16(f"ascr{v}") for v in range(7)]
        partials = S32("partials", 7)
        banks = [E(nc.psum_tensor(f"bank{v}", [128, F], f32))
                 for v in range(7)]

        sem_dma = E(nc.semaphore("sem_dma"))
        sem_ve = E(nc.semaphore("sem_ve"))      # milestones: z=1 eu/ev=2 wth=3
        sem_vp = E(nc.semaphore("sem_vp"))      # VE product stream
        sem_gp = E(nc.semaphore("sem_gp"))      # GP: c1=1 c2=2 products 3..8
        sem_act = E(nc.semaphore("sem_act"))    # r0=1, reduces 2..8
        sem_pe = E(nc.semaphore("sem_pe"))      # one per matmul

        theta = [crs[0], crs[1], crs[2], dts[0], dts[1], dts[2]]
        pr_pairs = [(gw, theta[i]) for i in range(6)] + [
            (wth[i], theta[j]) for (i, j) in HPAIRS
        ]
        pr_pairs.append(pr_pairs[0])  # pad to 28 so every bank fills
        GPK = [21, 22, 23, 24, 25, 26]   # dt-only hessian pairs -> GpSimd
        VEK = [k for k in range(NPROD) if k not in GPK]

        blk = E(nc.Block())

        @blk.sync
        def _(sync):
            sync.dma_start(blob1[:], in1_d[:]).then_inc(sem_dma, 16)
            sync.dma_start(blob2[:], in2_d[:]).then_inc(sem_dma, 16)
            sync.wait_ge(sem_act, 8)
            sync.dma_start(out_d[:], partials[:]).then_inc(sem_dma, 16)

        @blk.vector
        def _(vector):
            V = nc.vector

            def lin3(out, i0, i1, i2, k0, k1, k2, kt):
                # out = i0*c(k0) + i1*c(k1) + i2*c(k2) + c(kt)
                V.tensor_scalar(scr1[:], i0, c(k0), None, Alu.mult)
                V.tensor_scalar(scr2[:], i1, c(k1), None, Alu.mult)
                V.tensor_tensor(scr3[:], scr1[:], scr2[:], Alu.add)
                V.tensor_scalar(scr1[:], i2, c(k2), c(kt), Alu.mult, Alu.add)
                V.tensor_tensor(out, scr3[:], scr1[:], Alu.add)

            vector.wait_ge(sem_dma, 16)
            # z row first: feeds the ScalarE reciprocal
            lin3(z[:], px, py, pz, 6, 7, 8, 14)
            nc.vector.engine_nop().then_inc(sem_ve, 1)
            lin3(x[:], px, py, pz, 0, 1, 2, 12)
            lin3(y[:], px, py, pz, 3, 4, 5, 13)

            vector.wait_ge(sem_dma, 32)
            V.tensor_scalar(a[:], n0, c(15), None, Alu.mult)
            V.tensor_scalar(b[:], n1, c(16), None, Alu.mult)
            V.tensor_scalar(eu[:], ciu, -1.0, c(17), Alu.mult, Alu.add)
            V.tensor_scalar(
                ev[:], civ, -1.0, c(18), Alu.mult, Alu.add
            ).then_inc(sem_ve, 1)  # =2: eu/ev ready for GpSimd c1/c2
            V.tensor_scalar(w0[:], unc_, float(wscale), None, Alu.mult)
            V.tensor_tensor(w[:], w0[:], msk, Alu.mult)

            # Newton polish of the ScalarE spline reciprocal
            vector.wait_ge(sem_act, 1)
            V.tensor_tensor(zr[:], z[:], r0[:], Alu.mult)
            V.tensor_scalar(u2t[:], zr[:], -1.0, 2.0, Alu.mult, Alu.add)
            V.tensor_tensor(zinv32[:], r0[:], u2t[:], Alu.mult)
            V.tensor_copy(zinv[:], zinv32[:])
            V.tensor_tensor(zinv2[:], zinv[:], zinv[:], Alu.mult)

            V.tensor_tensor(dxt[:], a[:], zinv[:], Alu.mult)
            V.tensor_tensor(dyt[:], b[:], zinv[:], Alu.mult)
            V.tensor_tensor(t1[:], a[:], x[:], Alu.mult)
            V.tensor_tensor(t2[:], b[:], y[:], Alu.mult)
            V.tensor_tensor(num[:], t1[:], t2[:], Alu.add)
            V.tensor_tensor(dzp[:], num[:], zinv2[:], Alu.mult)
            V.tensor_tensor(pd[:], num[:], zinv[:], Alu.mult)
            V.tensor_tensor(md[:], mean_, pd[:], Alu.subtract)
            vector.wait_ge(sem_gp, 3)   # c1+c2 pre-sum from GpSimd
            V.tensor_tensor(md[:], md[:], gscr[:], Alu.subtract)
            V.tensor_tensor(gw[:], md[:], w[:], Alu.mult)

            for i in range(3):
                V.tensor_scalar(scr1[:], dxt[:], c(i), None, Alu.mult)
                V.tensor_scalar(scr2[:], dyt[:], c(3 + i), None, Alu.mult)
                V.tensor_tensor(scr3[:], scr1[:], scr2[:], Alu.add)
                V.tensor_scalar(scr1[:], dzp[:], c(9 + i), None, Alu.mult)
                V.tensor_tensor(dts[i][:], scr3[:], scr1[:], Alu.add)

            for i in range(3, 6):
                V.tensor_tensor(wth[i][:], w[:], theta[i][:], Alu.mult)
            nc.vector.engine_nop().then_inc(sem_ve, 1)  # =3: dt wth ready

            for (ri, ia, ib, pa, pb_) in (
                (0, 2, 1, py, pz),
                (1, 0, 2, pz, px),
                (2, 1, 0, px, py),
            ):
                V.tensor_tensor(scr1[:], pa, dts[ia][:], Alu.mult)
                V.tensor_tensor(scr2[:], pb_, dts[ib][:], Alu.mult)
                V.tensor_tensor(crs[ri][:], scr1[:], scr2[:], Alu.subtract)

            for i in range(3):
                V.tensor_tensor(wth[i][:], w[:], theta[i][:], Alu.mult)

            for k in VEK:
                ta, tb = pr_pairs[k]
                V.tensor_tensor(
                    prods[k][:], ta[:], tb[:], Alu.mult
                ).then_inc(sem_vp, 1)

        @blk.scalar
        def _(scalar):
            S = nc.scalar
            scalar.wait_ge(sem_ve, 1)
            S.add_instruction(
                mybir.InstActivation(
                    name=nc.get_next_instruction_name(),
                    func=AF.Reciprocal,
                    ins=[
                        S.lower_ap(z[:]),
                        mybir.ImmediateValue(dtype=f32, value=0.0),
                        mybir.ImmediateValue(dtype=f32, value=1.0),
                        mybir.ImmediateValue(dtype=f32, value=0.0),
                    ],
                    outs=[S.lower_ap(r0[:])],
                )
            ).then_inc(sem_act, 1)
            for v in range(7):
                scalar.wait_ge(sem_pe, 4 * v + 4)
                S.activation(
                    ascr[v][:], banks[v][:], AF.Copy,
                    accum_out=partials[:, v : v + 1],
                ).then_inc(sem_act, 1)

        @blk.gpsimd
        def _(gpsimd):
            G = nc.gpsimd
            gpsimd.wait_ge(sem_dma, 32)
            gpsimd.wait_ge(sem_ve, 2)
            G.tensor_tensor(c1t[:], n0, eu[:], Alu.mult).then_inc(sem_gp, 1)
            G.tensor_tensor(c2t[:], n1, ev[:], Alu.mult).then_inc(sem_gp, 1)
            G.tensor_tensor(gscr[:], c1t[:], c2t[:], Alu.add
                            ).then_inc(sem_gp, 1)
            gpsimd.wait_ge(sem_ve, 3)
            for k in GPK:
                ta, tb = pr_pairs[k]
                G.tensor_tensor(
                    prods[k][:], ta[:], tb[:], Alu.mult
                ).then_inc(sem_gp, 1)

        @blk.tensor
        def _(tensor):
            tensor.wait_ge(sem_dma, 32)
            ve_pos = {k: i + 1 for i, k in enumerate(VEK)}
            gp_pos = {k: i + 4 for i, k in enumerate(GPK)}
            for k in range(NPROD):
                if k in gp_pos:
                    tensor.wait_ge(sem_gp, gp_pos[k])
                else:
                    tensor.wait_ge(sem_vp, ve_pos[k])
                wv, j = k // 4, k % 4
                nc.tensor.matmul(
                    banks[wv][32 * j : 32 * j + 32, :],
                    sel,
                    prods[k][:],
                    start=True,
                    stop=True,
                    tile_position=(0, 32 * j),
                ).then_inc(sem_pe, 1)

    return nc


def _get_nc(cfg_key):
    if cfg_key not in _BUILT:
        _BUILT[cfg_key] = _build(CFG)
    return _BUILT[cfg_key]


def _pack_core_inputs(c, normals_in_image, centers_in_image, centers_in_body,
                      deformed_body2view_pose_data, camera_data,
                      valid_data_line, distribution_mean,
                      distribution_uncertainties, np_dt):
    sl = slice(PB * c, PB * (c + 1))

    def pack(arr2d):  # [PB, N] -> [128, F]
        return np.ascontiguousarray(
            arr2d.reshape(PB * PP, F), dtype=np_dt
        )

    p3 = centers_in_body[sl]
    nrm = normals_in_image[sl]
    cim = centers_in_image[sl]
    p3_a = np.concatenate([pack(p3[..., i]) for i in range(3)], axis=1)
    nrm_a = np.concatenate([pack(nrm[..., i]) for i in range(2)], axis=1)
    cim_a = np.concatenate([pack(cim[..., i]) for i in range(2)], axis=1)
    stat_a = np.concatenate(
        [
            pack(distribution_mean[sl]),
            pack(distribution_uncertainties[sl]),
            pack(valid_data_line[sl].astype(np.float32)),
        ],
        axis=1,
    )

    pose = deformed_body2view_pose_data[sl].astype(np.float32)  # [PB,12]
    cam = camera_data[sl].astype(np.float32)  # [PB,6]
    R = pose[:, :9]
    cstv = np.concatenate(
        [
            R,                               # 0..8
            -R[:, 6:9],                      # 9..11  (-R[2,:])
            pose[:, 9:12],                   # 12..14 (t)
            cam[:, 2:3], cam[:, 3:4],        # 15, 16 (fu, fv)
            cam[:, 4:5], cam[:, 5:6],        # 17, 18 (cu, cv)
            np.zeros((PB, 1), np.float32),   # 19 (zero bias)
        ],
        axis=1,
    )  # [PB, 20]
    cst_a = np.ascontiguousarray(np.repeat(cstv, PP, axis=0), dtype=np.float32)

    selw = PB if CFG["reduce"] != "pe" else 32
    sel_np = np_dt if CFG["reduce"] == "pe" else np.float32
    sel_a = np.zeros((128, selw), sel_np)
    for j in range(selw):
        sel_a[PP * (j % PB) : PP * (j % PB + 1), j] = 1.0

    if np_dt == np.float16:
        cst_cols = np.ascontiguousarray(cst_a).view(np.float16)
    else:
        cst_cols = cst_a
    in1 = np.concatenate([cst_cols.astype(cst_cols.dtype), p3_a], axis=1)
    sel_cols = sel_a
    if np_dt == np.float16 and sel_a.dtype == np.float32:
        sel_cols = np.ascontiguousarray(sel_a).view(np.float16)
    in2 = np.concatenate([nrm_a, cim_a, stat_a, sel_cols], axis=1)
    return {"in1": np.ascontiguousarray(in1),
            "in2": np.ascontiguousarray(in2)}


def _decode(results, cfg):
    wscale = 2.0 ** -10 if cfg["dtype"] == "f16" else 1.0
    parts = np.zeros((B, NQ), np.float32)
    for cidx in range(NCORES):
        out = np.asarray(results[cidx]["out"], np.float32)
        if cfg["reduce"] == "pe":
            for q in range(NQ):
                wv, j = q // 4, q % 4
                parts[PB * cidx : PB * (cidx + 1), q] = out[
                    32 * j : 32 * j + PB, wv
                ]
        else:
            parts[PB * cidx : PB * (cidx + 1), :] = out
    parts /= wscale
    grad = np.ascontiguousarray(parts[:, :6, None], np.float32)
    hess = np.zeros((B, 6, 6), np.float32)
    for idx, (i, j) in enumerate(HPAIRS):
        hess[:, i, j] = parts[:, 6 + idx]
        hess[:, j, i] = parts[:, 6 + idx]
    return grad, hess


def kernel(normals_in_image, centers_in_image, centers_in_body,
           deformed_body2view_pose_data, camera_data, valid_data_line,
           distributions=None, distribution_mean=None,
           distribution_uncertainties=None, it=None, **_unused):
    from concourse.bass_utils import run_bass_kernel_spmd

    cfg = CFG
    np_dt = np.float32 if cfg["dtype"] == "f32" else np.float16
    nc = _get_nc((cfg["dtype"], cfg["reduce"]))

    args = (
        np.asarray(normals_in_image),
        np.asarray(centers_in_image),
        np.asarray(centers_in_body),
        np.asarray(deformed_body2view_pose_data),
        np.asarray(camera_data),
        np.asarray(valid_data_line),
        np.asarray(distribution_mean),
        np.asarray(distribution_uncertainties),
    )
    in_maps = [_pack_core_inputs(c, *args, np_dt) for c in range(NCORES)]
    res = run_bass_kernel_spmd(
        nc, in_maps, core_ids=list(range(NCORES)), **TRACE_KW
    )
    global LAST_RESULTS
    LAST_RESULTS = res
    return _decode(res.results, cfg)
